# revision 1
# baseline (speedup 1.0000x reference)
"""Trainium2 Bass kernel v2: batched single-head attention + residual + layernorm.

Per batch element b (one NeuronCore each, data-parallel over B=8):
    q = X@Wq+bq; k = X@Wk+bk; v = X@Wv+bv          [S=2048, K=64]
    attn = softmax(q @ k.T / 8, axis=-1)            [S, S]
    y = X + (attn @ v) @ Wo + bo                    [S, D=1024]
    out = layernorm(y) * gamma + beta

Cost-model-driven v2 design (vs v1 baseline):
  - Host passes X twice in compact layouts: X.T in fp8e4m3 (projections) and
    X in bf16 (residual). This removes all on-chip X transposes (PE) and the
    PSUM->SBUF transpose copies (DVE), and halves X DMA traffic.
  - Projections run as fp8 DoubleRow matmuls (2 d-chunks per matmul via 3D
    [128,2,*] APs) -> PSUM f32; bias applied by the PSUM->SBUF copy.
  - Scores: bf16 matmuls k_tile.T @ q (k duplicated to partitions 0:64 by a
    tiny SBUF->SBUF DMA); exp on ScalarE reads 1024-wide (2 sk-tiles per
    instruction) writing fp8 expT.
  - attn@v: fp8 DoubleRow over sk-tile pairs; an extra v column of 1/64
    accumulates sums/64 in the same PSUM group. recip -> 64/sums broadcast by
    a PE ones-matmul into transient PSUM; av8 = uav * (64/sums) in fp8.
  - y is computed scaled by 512 (layernorm is scale-invariant; eps scaled to
    match): psy = (64 av) @ (8 Wo) [fp8 DoubleRow via avT8 [33,2,S] built with
    two small DMAs] + (512 I) @ X_bf16. LN stats via bn_stats/bn_aggr, rstd
    via ACT sqrt + DVE reciprocal; normalize split DVE/ACT per 512-half;
    output stored bf16 (host upcasts to f32).
  - Output stores + small copies routed through the idle GPSIMD DMA queue.

gamma/beta are ones/zeros for this problem; applied on host if non-trivial.
"""

import numpy as np

B = 8
S = 2048
D = 1024
K = 64
EPS = 1e-5

NT = S // 128   # 16 s-tiles
NC_ = D // 128  # 8 d-chunks
NB = S // 512   # 4 query blocks

YS = 512.0      # psy = 512*y
AVS = 64.0      # av8 = 64*av ; v sums col = 1/64
WOS = 8.0       # wob8 = 8*Wo rows

_COMPILED = {}


def _build_bass(act_norm_tiles=8, taps=False):
    import concourse.bacc as bacc
    import concourse.tile as tile
    from concourse import mybir
    from concourse.masks import make_identity

    f32 = mybir.dt.float32
    f32r = mybir.dt.float32r
    bf16 = mybir.dt.bfloat16
    f8 = mybir.dt.float8e4
    AF = mybir.ActivationFunctionType
    DR = mybir.MatmulPerfMode.DoubleRow

    nc = bacc.Bacc("TRN2", target_bir_lowering=False, debug=False)

    xb_dram = nc.dram_tensor("XB", [S, D], bf16, kind="ExternalInput")
    xt8_dram = nc.dram_tensor("XT8", [D, S], f8, kind="ExternalInput")
    wqk8_dram = nc.dram_tensor("WQK8", [D, 128], f8, kind="ExternalInput")
    wv8_dram = nc.dram_tensor("WV8", [D, K], f8, kind="ExternalInput")
    bqk_dram = nc.dram_tensor("BQK", [128], f32, kind="ExternalInput")
    bv_dram = nc.dram_tensor("BV", [K], bf16, kind="ExternalInput")
    wob8_dram = nc.dram_tensor("WOB8", [33, 2, D], f8, kind="ExternalInput")
    out_dram = nc.dram_tensor("OUT", [S, D], bf16, kind="ExternalOutput")
    tap_handles = {}
    if taps:
        for name, shape, dt_ in [
            ("T_QK", [K, 2, S], mybir.dt.bfloat16),
            ("T_V", [128, NT, K + 1], mybir.dt.float8e4),
            ("T_EXP0", [128, NT, 512], mybir.dt.float8e4),
            ("T_AVT", [33, 2, S], mybir.dt.float8e4),
            ("T_RECB", [K, 512], mybir.dt.float32),
        ]:
            tap_handles[name] = nc.dram_tensor(name, shape, dt_, kind="ExternalOutput")

    with tile.TileContext(nc) as tc:
        with (
            tc.tile_pool(name="consts", bufs=1) as consts,
            tc.tile_pool(name="bigx", bufs=1) as bigx,
            tc.tile_pool(name="proj", bufs=1) as proj,
            tc.tile_pool(name="vtp", bufs=2) as vtp,
            tc.tile_pool(name="avn", bufs=2) as avn,
            tc.tile_pool(name="outp", bufs=3) as outp,
            tc.tile_pool(name="work", bufs=4) as work,
            tc.tile_pool(name="expp", bufs=2) as expp,
            tc.tile_pool(name="psS", bufs=2, space="PSUM") as psS,
            tc.tile_pool(name="psU", bufs=1, space="PSUM") as psU,
        ):
            # Pre-place the act table that serves Exp+Ln+Identity+Copy so the
            # compiler's table-load pass doesn't flip-flop between the
            # exp-only and ln-only tables (1283ns per reload).
            nc.scalar.add_instruction(
                mybir.InstLoadActFuncSet(
                    name=nc.get_next_instruction_name(),
                    ins=[], outs=[], act_func_set_id=6,
                )
            )
            ident = consts.tile([128, 128], f32)
            make_identity(nc, ident)
            ident512 = consts.tile([128, 128], bf16)
            nc.scalar.mul(ident512, ident, YS)
            # Ln/Exp table inputs kept O(1): ln((512^2 var) * 2^-18 + eps)
            # = ln(var + eps); exp(-0.5*lnv - 9*ln2) = (512^2 var + eps')^-0.5
            epsS_t = consts.tile([128, 1], f32)
            nc.vector.memset(epsS_t, EPS)
            nln2_t = consts.tile([128, 1], f32)
            nc.vector.memset(nln2_t, -9.0 * 0.6931471805599453)
            ones_f = consts.tile([128, 512], f32)
            nc.vector.memset(ones_f, 1.0)
            ones_bc = consts.tile([128, K], f32r)  # row 64 used as [1,64]
            with nc.allow_low_precision(reason="ones constant to f32r"):
                nc.scalar.copy(out=ones_bc, in_=ones_f[:, 0:K])
            ones512r = consts.tile([1, 512], f32r)
            nc.scalar.copy(out=ones512r, in_=ones_f[0:1, :])
            ones_row128 = consts.tile([1, 128], bf16)
            nc.vector.memset(ones_row128, 1.0)
            bqk_row = consts.tile([1, 2, K], f32r)  # [q|k] bias as matmul lhsT

            # Load plan: HWDGE (SP queue) and SWDGE (Pool queue) generate
            # descriptors on separate devices, so split the critical xt8
            # chunks across both; weights follow on Pool; xb halves on each;
            # everything ordered so projection operands land first.
            xt8_sb = bigx.tile([128, NC_, S], f8)
            xt8_view = xt8_dram[:].rearrange("(c p) s -> p c s", p=128)
            xb_sb = bigx.tile([128, NT, D], bf16)
            xb_view = xb_dram[:].rearrange("(t p) d -> p t d", p=128)
            wqk8 = consts.tile([128, NC_, 128], f8)
            wv8 = consts.tile([128, NC_, K], f8)
            bqk_row = consts.tile([1, 2, K], f32r)  # [q|k] bias as matmul lhsT
            bv_row8 = consts.tile([1, K], bf16)
            wob8 = consts.tile([33, 2, D], f8)
            # Pool/SWDGE side: weights first (parallel desc-gen with SP's
            # xt8 HWDGE gens), then all of xb, then output weights
            nc.gpsimd.dma_start(
                out=wqk8, in_=wqk8_dram[:].rearrange("(c p) m -> p c m", p=128)
            )
            nc.gpsimd.dma_start(
                out=wv8, in_=wv8_dram[:].rearrange("(c p) k -> p c k", p=128)
            )
            nc.gpsimd.dma_start(
                out=bqk_row,
                in_=bqk_dram[:].rearrange("(a j k) -> a j k", a=1, j=2).bitcast(f32r),
            )
            nc.gpsimd.dma_start(
                out=bv_row8, in_=bv_dram[:].rearrange("(a k) -> a k", a=1)
            )
            nc.gpsimd.dma_start(out=wob8, in_=wob8_dram[:])
            # SP/HWDGE side: the 8 critical xt8 chunks, nothing else ahead
            for c in range(NC_):
                nc.sync.dma_start(out=xt8_sb[:, c, :], in_=xt8_view[:, c, :])
            for t in range(NT):
                nc.gpsimd.dma_start(out=xb_sb[:, t, :], in_=xb_view[:, t, :])

            qk2_sb = proj.tile([K, 2, S], bf16)  # [:,0,:] q, [:,1,:] k
            v_sb = proj.tile([128, NT, 80], f8)  # cols 0:64 v, col 64 = 1/64
            nc.gpsimd.memset(v_sb[:, :, K : K + 1], 1.0 / AVS)
            avT8 = proj.tile([33, 2, S], f8)
            nc.gpsimd.memset(avT8[32:33, 0, :], AVS)
            nc.gpsimd.memset(avT8[32:33, 1, :], 0.0)
            util = proj.tile([128, 512], f32r)  # row 64: recip1 scratch

            exp_tiles = {}

            def emit_scores(tgt, pair_list):
                if tgt not in exp_tiles:
                    et = expp.tile([128, NT, 512], f8, tag="expT", name=f"expT{tgt}")
                    exp_tiles[tgt] = et
                et = exp_tiles[tgt]
                sqt = slice(tgt * 512, (tgt + 1) * 512)
                for p in pair_list:
                    pss = psS.tile([128, 2, 512], f32, tag="pss", name=f"pss{tgt}_{p}")
                    for j in range(2):
                        sk = 2 * p + j
                        nc.tensor.matmul(
                            pss[:, j, :],
                            qk2_sb[:, 1, sk * 128 : (sk + 1) * 128],
                            qk2_sb[:, 0, sqt],
                            start=True,
                            stop=True,
                        )
                    nc.scalar.activation(
                        out=et[:, 2 * p : 2 * p + 2, :], in_=pss[:],
                        func=AF.Exp, scale=0.125,
                    )

            # ---- phase 1: projections; block-0 scores piped in, block-1
            # scores emitted after (so expT(0) completes as early as possible
            # and phase 2 can start under the block-1 exp stream) ----
            with tc.tile_pool(name="psP", bufs=3, space="PSUM") as psP:
                for b in range(NB):
                    sq = slice(b * 512, (b + 1) * 512)
                    # q and k as separate 64-col chains, single-bank tiles
                    # (both land on partitions 0:64; DVE copies cannot cross
                    # partitions); biases folded in as ones-row matmuls
                    for j, tg in ((0, "psq"), (1, "psk")):
                        psj = psP.tile([K, 512], f32, tag=tg, bufs=1)
                        for cp in range(4):
                            nc.tensor.matmul(
                                psj,
                                wqk8[:, 2 * cp : 2 * cp + 2, j * K : (j + 1) * K],
                                xt8_sb[:, 2 * cp : 2 * cp + 2, sq],
                                start=(cp == 0),
                                stop=False,
                                perf_mode=DR,
                            )
                        nc.tensor.matmul(
                            psj,
                            bqk_row[:, j, :],
                            ones512r,
                            start=False,
                            stop=True,
                        )
                        nc.vector.tensor_copy(out=qk2_sb[:, j, sq], in_=psj)
                    # v in natural [s, j] layout: xt8 chunks stationary,
                    # wv8 chunk moving; bias via a ones-row matmul
                    psv = psP.tile([128, 4, K], f32, tag="psv", bufs=1)
                    for ti in range(4):
                        t = b * 4 + ti
                        for c in range(NC_):
                            nc.tensor.matmul(
                                psv[:, ti, :],
                                xt8_sb[:, c, t * 128 : (t + 1) * 128],
                                wv8[:, c, :],
                                start=(c == 0),
                                stop=False,
                            )
                        nc.tensor.matmul(
                            psv[:, ti, :],
                            ones_row128,
                            bv_row8,
                            start=False,
                            stop=True,
                        )
                    nc.vector.tensor_copy(
                        out=v_sb[:, b * 4 : (b + 1) * 4, 0:K], in_=psv
                    )
                    # scores for block 0 as its k-tiles become available
                    emit_scores(0, range(b * 2, b * 2 + 2))

            # ---- phase 2 ----
            out_view = out_dram[:].rearrange("(t p) d -> p t d", p=128)

            def emit_av(b, splits=1):
                """uav -> recip -> bcast -> av8 -> avT8 DMAs for block b.
                splits>1 pipelines the post-uav chain in sq-subslices so the
                first y-tiles of the block unblock after 1/splits of the
                chain (used for the cold block-0 seam)."""
                expT = exp_tiles.pop(b)
                psu = psU.tile([K + 1, 512], f32, tag="psu", name=f"psu{b}")
                for tp in range(NT // 2):
                    nc.tensor.matmul(
                        psu,
                        v_sb[:, 2 * tp : 2 * tp + 2, 0 : K + 1],
                        expT[:, 2 * tp : 2 * tp + 2, :],
                        start=(tp == 0),
                        stop=(tp == NT // 2 - 1),
                        perf_mode=DR,
                    )
                psbct = psS.tile([128, 2, 512], f32, tag="pss", name=f"psbc{b}")
                recipb = avn.tile([K, 512], f32, tag="recipb")
                av8 = avn.tile([K, 512], f8, tag="av8")
                w = 512 // splits
                for s in range(splits):
                    ss = slice(s * w, (s + 1) * w)
                    sq = slice(b * 512 + s * w, b * 512 + (s + 1) * w)
                    # recip = 64/sums at util row 64; broadcast to 64
                    # partitions via a f32r ones-matmul into a transient psS
                    # slot, then copy to SBUF so the av multiply reads only
                    # one PSUM operand
                    with nc.allow_low_precision(reason="recip feeds f32r bcast"):
                        nc.vector.reciprocal(
                            out=util[K : K + 1, ss], in_=psu[K : K + 1, ss]
                        )
                    psbc = psbct[0:K, 0, ss]
                    nc.tensor.matmul(
                        psbc,
                        ones_bc[K : K + 1, :],
                        util[K : K + 1, ss],
                        start=True,
                        stop=True,
                    )
                    nc.vector.tensor_copy(out=recipb[:, ss], in_=psbc)
                    nc.vector.tensor_tensor(
                        out=av8[:, ss], in0=psu[0:K, ss], in1=recipb[:, ss],
                        op=mybir.AluOpType.mult,
                    )
                    nc.sync.dma_start(out=avT8[0:32, 0, sq], in_=av8[0:32, ss])
                    nc.sync.dma_start(out=avT8[0:32, 1, sq], in_=av8[32:K, ss])
                if taps and b == 0:
                    nc.gpsimd.dma_start(out=tap_handles["T_RECB"][:], in_=recipb)

            if taps:
                nc.gpsimd.dma_start(out=tap_handles["T_QK"][:], in_=qk2_sb[:])
                nc.gpsimd.dma_start(out=tap_handles["T_V"][:], in_=v_sb[:, :, 0 : K + 1])
            # Software-pipelined LN: at iteration t, the j=1 half is normalized
            # on ACT (same engine as rstd, no cross-engine wait); the j=0 half
            # of iteration t-1 is normalized on DVE using the then-ready rstd,
            # so the in-order DVE queue never waits on ACT.
            #
            # Remaining score-pairs (blocks 1-3) are fed from a global queue,
            # 2 per tile iteration AFTER that tile's y-work, so the in-order
            # PE queue never parks y matmuls behind exp-paced score matmuls.
            pair_queue = [(tgt, p) for tgt in range(1, NB) for p in range(NT // 2)]

            def emit_next_pairs(n):
                for _ in range(n):
                    if pair_queue:
                        tgt, p = pair_queue.pop(0)
                        emit_scores(tgt, [p])

            with tc.tile_pool(name="psY", bufs=3, space="PSUM") as psY:
                if taps:
                    nc.gpsimd.dma_start(
                        out=tap_handles["T_EXP0"][:], in_=exp_tiles[0][:]
                    )
                emit_av(0, splits=2)
                emit_next_pairs(4)  # seam pre-fill while av(0) round-trips
                prev = None  # (psy0, out_sb, mv, rstd, t)
                for b in range(NB):
                    if taps and b == NB - 1:
                        nc.gpsimd.dma_start(out=tap_handles["T_AVT"][:], in_=avT8[:])
                    for ti in range(4):
                        t = b * 4 + ti
                        out_sb = outp.tile([128, D], bf16, tag="o")
                        psy = [None, None]
                        stats = work.tile([128, 2, 6], f32, tag="stats")
                        # block 3: the score-psum pool is free; use its 2-bank
                        # tiles for y so stats/norm run 1024-wide and the LN
                        # pipeline gets extra depth
                        psyt = None
                        if (b == NB - 1 and ti % 2 == 0) or t == 11:
                            psyt = psS.tile(
                                [128, 2, 512], f32, tag="pss", name=f"psy2_{t}"
                            )
                        for j in range(2):
                            if psyt is not None:
                                psy_j = psyt[:, j, :]
                            else:
                                psy_j = psY.tile([128, 512], f32, tag="ps")
                            psy[j] = psy_j
                            nc.tensor.matmul(
                                psy_j,
                                avT8[:, :, t * 128 : (t + 1) * 128],
                                wob8[:, :, j * 512 : (j + 1) * 512],
                                start=True,
                                stop=False,
                                perf_mode=DR,
                            )
                            nc.tensor.matmul(
                                psy_j,
                                ident512,
                                xb_sb[:, t, j * 512 : (j + 1) * 512],
                                start=False,
                                stop=True,
                            )
                            nc.vector.bn_stats(out=stats[:, j, :], in_=psy_j)
                        mv = work.tile([128, 2], f32, tag="mv")
                        nc.vector.bn_aggr(out=mv, in_=stats)
                        mneg = work.tile([128, 1], f32, tag="mneg")
                        nc.vector.tensor_scalar(
                            out=mneg, in0=mv[:, 0:1], scalar1=-1.0, scalar2=None,
                            op0=mybir.AluOpType.mult,
                        )
                        # rstd = (var+eps)^-0.5 = exp(-0.5*ln(var+eps)); Ln and
                        # Exp share act table 6, so no reloads vs softmax exp.
                        lnv = work.tile([128, 1], f32, tag="lnv")
                        nc.scalar.activation(
                            out=lnv, in_=mv[:, 1:2], func=AF.Ln,
                            bias=epsS_t, scale=1.0 / (YS * YS),
                        )
                        rstd = work.tile([128, 1], f32, tag="rstd")
                        nc.scalar.activation(
                            out=rstd, in_=lnv, func=AF.Exp, scale=-0.5,
                            bias=nln2_t,
                        )
                        # nm = -mu*rstd, on ACT so the chain stays ACT-local
                        nm = work.tile([128, 1], f32, tag="nm")
                        nc.scalar.activation(
                            out=nm, in_=mneg, func=AF.Copy, scale=rstd,
                        )
                        if psyt is None:
                            nc.scalar.activation(
                                out=out_sb[:, 512:1024], in_=psy[1],
                                func=AF.Identity, bias=nm, scale=rstd,
                            )
                            if t == NT - 1:
                                # last tile: store the j1 half as soon as it
                                # is normalized so its DMA pipeline overlaps
                                # the parallel j0 norm
                                nc.sync.dma_start(
                                    out=out_view[:, t, 512:1024],
                                    in_=out_sb[:, 512:1024],
                                )
                        if prev is not None:
                            p_psy0, p_out, p_mv, p_rstd, p_t = prev
                            nc.vector.tensor_scalar(
                                out=p_out[:, 0:512], in0=p_psy0,
                                scalar1=p_mv[:, 0:1], scalar2=p_rstd,
                                op0=mybir.AluOpType.subtract,
                                op1=mybir.AluOpType.mult,
                            )
                            nc.sync.dma_start(
                                out=out_view[:, p_t, :], in_=p_out
                            )
                            prev = None
                        if b == NB - 1 or psyt is not None:
                            # last block: exps are done, ACT has slack and DVE
                            # is the bottleneck -> norm j0 on ACT in-iteration
                            # (psS-based tiles normalize 1024-wide in one op,
                            # so skip the separate j1 norm below for them)
                            if psyt is not None:
                                nc.scalar.activation(
                                    out=out_sb[:].rearrange(
                                        "p (j d) -> p j d", j=2
                                    ),
                                    in_=psyt,
                                    func=AF.Identity, bias=nm, scale=rstd,
                                )
                            elif t >= NT - 3 and psyt is None:
                                # very last tile: nothing queued behind it on
                                # DVE, so run j0 there in parallel with ACT's
                                # j1, and store each half as soon as it is
                                # normalized (separate queues)
                                nc.vector.tensor_scalar(
                                    out=out_sb[:, 0:512], in0=psy[0],
                                    scalar1=mv[:, 0:1], scalar2=rstd,
                                    op0=mybir.AluOpType.subtract,
                                    op1=mybir.AluOpType.mult,
                                )
                            else:
                                nc.scalar.activation(
                                    out=out_sb[:, 0:512], in_=psy[0],
                                    func=AF.Identity, bias=nm, scale=rstd,
                                )
                            if t == NT - 1:
                                nc.sync.dma_start(
                                    out=out_view[:, t, 0:512],
                                    in_=out_sb[:, 0:512],
                                )
                            else:
                                nc.sync.dma_start(
                                    out=out_view[:, t, :], in_=out_sb
                                )
                        else:
                            prev = (psy[0], out_sb, mv, rstd, t)
                        if ti == 2 and b + 1 < NB:
                            emit_av(b + 1)
                        emit_next_pairs(2)
                # tail: finish any pending deferred tile
                if prev is not None:
                    p_psy0, p_out, p_mv, p_rstd, p_t = prev
                    nc.vector.tensor_scalar(
                        out=p_out[:, 0:512], in0=p_psy0,
                        scalar1=p_mv[:, 0:1], scalar2=p_rstd,
                        op0=mybir.AluOpType.subtract,
                        op1=mybir.AluOpType.mult,
                    )
                    nc.sync.dma_start(out=out_view[:, p_t, :], in_=p_out)

    nc.compile()
    return nc


def _get_compiled():
    if "nc" not in _COMPILED:
        _COMPILED["nc"] = _build_bass()
    return _COMPILED["nc"]


def _host_inputs(X, Wq, bq, Wk, bk, Wv, bv, Wo, bo):
    import ml_dtypes

    f8 = ml_dtypes.float8_e4m3
    bf = ml_dtypes.bfloat16
    f32 = np.float32

    wqk8 = np.ascontiguousarray(
        np.concatenate([Wq, Wk], axis=1).astype(f32)
    ).astype(f8)
    wv8 = np.ascontiguousarray(Wv.astype(f32)).astype(f8)
    bqk = np.concatenate([bq, bk]).astype(f32)
    wob8 = np.zeros((33, 2, D), dtype=f8)
    wo8 = (Wo.astype(f32) * WOS).astype(f8)
    wob8[:32, 0, :] = wo8[0:32]
    wob8[:32, 1, :] = wo8[32:64]
    wob8[32, 0, :] = (bo.astype(f32) * WOS).astype(f8)

    common = {
        "WQK8": wqk8,
        "WV8": wv8,
        "BQK": bqk,
        "BV": bv.astype(f32).astype(bf),
        "WOB8": wob8,
    }
    per_core = []
    for i in range(X.shape[0]):
        Xi = np.ascontiguousarray(X[i], dtype=f32)
        per_core.append(
            {
                "XB": Xi.astype(bf),
                "XT8": np.ascontiguousarray(Xi.T).astype(f8),
                **common,
            }
        )
    return per_core


def kernel(X, Wq, bq, Wk, bk, Wv, bv, Wo, bo, gamma, beta):
    from concourse.bass_utils import run_bass_kernel_spmd

    X = np.asarray(X, dtype=np.float32)
    gamma_np = np.asarray(gamma, dtype=np.float32)
    beta_np = np.asarray(beta, dtype=np.float32)

    nc = _get_compiled()
    in_maps = _host_inputs(
        X,
        np.asarray(Wq), np.asarray(bq), np.asarray(Wk), np.asarray(bk),
        np.asarray(Wv), np.asarray(bv), np.asarray(Wo), np.asarray(bo),
    )
    res = run_bass_kernel_spmd(nc, in_maps, core_ids=list(range(B)))
    out = np.stack(
        [np.asarray(res.results[i]["OUT"]).astype(np.float32) for i in range(B)],
        axis=0,
    )
    if not (np.all(gamma_np == 1.0) and np.all(beta_np == 0.0)):
        out = out * gamma_np + beta_np
    return out.astype(np.float32)



# revision 6
# speedup vs baseline: 1.0109x; 1.0109x over previous
"""Trainium2 Bass kernel v2: batched single-head attention + residual + layernorm.

Per batch element b (one NeuronCore each, data-parallel over B=8):
    q = X@Wq+bq; k = X@Wk+bk; v = X@Wv+bv          [S=2048, K=64]
    attn = softmax(q @ k.T / 8, axis=-1)            [S, S]
    y = X + (attn @ v) @ Wo + bo                    [S, D=1024]
    out = layernorm(y) * gamma + beta

Cost-model-driven v2 design (vs v1 baseline):
  - Host passes X twice in compact layouts: X.T in fp8e4m3 (projections) and
    X in bf16 (residual). This removes all on-chip X transposes (PE) and the
    PSUM->SBUF transpose copies (DVE), and halves X DMA traffic.
  - Projections run as fp8 DoubleRow matmuls (2 d-chunks per matmul via 3D
    [128,2,*] APs) -> PSUM f32; bias applied by the PSUM->SBUF copy.
  - Scores: bf16 matmuls k_tile.T @ q (k duplicated to partitions 0:64 by a
    tiny SBUF->SBUF DMA); exp on ScalarE reads 1024-wide (2 sk-tiles per
    instruction) writing fp8 expT.
  - attn@v: fp8 DoubleRow over sk-tile pairs; an extra v column of 1/64
    accumulates sums/64 in the same PSUM group. recip -> 64/sums broadcast by
    a PE ones-matmul into transient PSUM; av8 = uav * (64/sums) in fp8.
  - y is computed scaled by 512 (layernorm is scale-invariant; eps scaled to
    match): psy = (64 av) @ (8 Wo) [fp8 DoubleRow via avT8 [33,2,S] built with
    two small DMAs] + (512 I) @ X_bf16. LN stats via bn_stats/bn_aggr, rstd
    via ACT sqrt + DVE reciprocal; normalize split DVE/ACT per 512-half;
    output stored bf16 (host upcasts to f32).
  - Output stores + small copies routed through the idle GPSIMD DMA queue.

gamma/beta are ones/zeros for this problem; applied on host if non-trivial.
"""

import numpy as np

B = 8
S = 2048
D = 1024
K = 64
EPS = 1e-5

NT = S // 128   # 16 s-tiles
NC_ = D // 128  # 8 d-chunks
NB = S // 512   # 4 query blocks

YS = 512.0      # psy = 512*y
AVS = 64.0      # av8 = 64*av ; v sums col = 1/64
WOS = 8.0       # wob8 = 8*Wo rows

_COMPILED = {}


def _build_bass(act_norm_tiles=8, taps=False):
    import concourse.bacc as bacc
    import concourse.tile as tile
    from concourse import mybir
    from concourse.masks import make_identity

    f32 = mybir.dt.float32
    f32r = mybir.dt.float32r
    bf16 = mybir.dt.bfloat16
    f8 = mybir.dt.float8e4
    AF = mybir.ActivationFunctionType
    DR = mybir.MatmulPerfMode.DoubleRow

    nc = bacc.Bacc("TRN2", target_bir_lowering=False, debug=False)

    xb_dram = nc.dram_tensor("XB", [S, D], bf16, kind="ExternalInput")
    xt8_dram = nc.dram_tensor("XT8", [D, S], f8, kind="ExternalInput")
    # all projection weights packed host-side into one [128, 8, 192] tensor:
    # cols 0:64 q, 64:128 k, 128:192 v per d-chunk -> ONE 546ns DMA with
    # 1536B-contiguous rows instead of 4 small strided loads.
    wqkv8_dram = nc.dram_tensor("WQKV8", [128, NC_, 192], f8, kind="ExternalInput")
    bqk_dram = nc.dram_tensor("BQK", [128], f32, kind="ExternalInput")
    bv_dram = nc.dram_tensor("BV", [K], bf16, kind="ExternalInput")
    wob8_dram = nc.dram_tensor("WOB8", [33, 2, D], f8, kind="ExternalInput")
    out_dram = nc.dram_tensor("OUT", [S, D], bf16, kind="ExternalOutput")
    tap_handles = {}
    if taps:
        for name, shape, dt_ in [
            ("T_QK", [K, 2, S], mybir.dt.bfloat16),
            ("T_V", [128, NT, K + 1], mybir.dt.float8e4),
            ("T_EXP0", [128, NT, 512], mybir.dt.float8e4),
            ("T_AVT", [33, 2, S], mybir.dt.float8e4),
            ("T_RECB", [K, 512], mybir.dt.float32),
        ]:
            tap_handles[name] = nc.dram_tensor(name, shape, dt_, kind="ExternalOutput")

    with tile.TileContext(nc) as tc:
        with (
            tc.tile_pool(name="consts", bufs=1) as consts,
            tc.tile_pool(name="bigx", bufs=1) as bigx,
            tc.tile_pool(name="proj", bufs=1) as proj,
            tc.tile_pool(name="vtp", bufs=2) as vtp,
            tc.tile_pool(name="avn", bufs=2) as avn,
            tc.tile_pool(name="outp", bufs=3) as outp,
            tc.tile_pool(name="work", bufs=4) as work,
            tc.tile_pool(name="expp", bufs=2) as expp,
            tc.tile_pool(name="psS", bufs=2, space="PSUM") as psS,
            tc.tile_pool(name="psU", bufs=1, space="PSUM") as psU,
        ):
            # Pre-place the act table that serves Exp+Ln+Identity+Copy so the
            # compiler's table-load pass doesn't flip-flop between the
            # exp-only and ln-only tables (1283ns per reload).
            nc.scalar.add_instruction(
                mybir.InstLoadActFuncSet(
                    name=nc.get_next_instruction_name(),
                    ins=[], outs=[], act_func_set_id=6,
                )
            )
            ident = consts.tile([128, 128], f32)
            make_identity(nc, ident)
            ident512 = consts.tile([128, 128], bf16)
            nc.gpsimd.tensor_scalar_mul(ident512, ident, YS)
            # Ln/Exp table inputs kept O(1): ln((512^2 var) * 2^-18 + eps)
            # = ln(var + eps); exp(-0.5*lnv - 9*ln2) = (512^2 var + eps')^-0.5
            epsS_t = consts.tile([128, 1], f32)
            nc.vector.memset(epsS_t, EPS)
            nln2_t = consts.tile([128, 1], f32)
            nc.vector.memset(nln2_t, -9.0 * 0.6931471805599453)
            ones_f = consts.tile([128, 512], f32)
            nc.gpsimd.memset(ones_f, 1.0)
            ones_bc = consts.tile([128, K], f32r)  # row 64 used as [1,64]
            with nc.allow_low_precision(reason="ones constant to f32r"):
                nc.gpsimd.tensor_copy(out=ones_bc, in_=ones_f[:, 0:K])
            ones512r = consts.tile([1, 512], f32r)
            with nc.allow_low_precision(reason="ones constant to f32r"):
                nc.gpsimd.tensor_copy(out=ones512r, in_=ones_f[0:1, :])
            ones_row128 = consts.tile([1, 128], bf16)
            nc.gpsimd.memset(ones_row128, 1.0)

            # Load plan: SP/HWDGE queue carries the packed weights then xt8 in
            # two 1MB transfers (minimal desc-gen serialization -> first q
            # matmul ~4us earlier than 8 chunked loads). Pool/SWDGE carries
            # the small biases + wob8 + xb (desc-gen on the otherwise-idle
            # Pool engine, transfers interleave behind xt8 on the DMA device).
            xt8_sb = bigx.tile([128, NC_, S], f8)
            xt8_view = xt8_dram[:].rearrange("(c p) s -> p c s", p=128)
            xb_sb = bigx.tile([128, NT, D], bf16)
            xb_view = xb_dram[:].rearrange("(t p) d -> p t d", p=128)
            wqkv8 = consts.tile([128, NC_, 192], f8)
            bqk_row = consts.tile([1, 2, K], f32r)  # [q|k] bias as matmul lhsT
            bv_row8 = consts.tile([1, K], bf16)
            wob8 = consts.tile([33, 2, D], f8)
            # SP/HWDGE side: weights then the two xt8 halves, nothing ahead
            nc.sync.dma_start(out=wqkv8, in_=wqkv8_dram[:])
            nc.sync.dma_start(out=xt8_sb[:, 0:4, :], in_=xt8_view[:, 0:4, :])
            nc.sync.dma_start(out=xt8_sb[:, 4:8, :], in_=xt8_view[:, 4:8, :])
            # Pool/SWDGE side: biases (needed ~t=6us), wob8, then xb in 8
            # 2-tile chunks (small enough to not hog the DMA device at the
            # avT8 round-trip seams)
            nc.gpsimd.dma_start(
                out=bqk_row,
                in_=bqk_dram[:].rearrange("(a j k) -> a j k", a=1, j=2).bitcast(f32r),
            )
            nc.gpsimd.dma_start(
                out=bv_row8, in_=bv_dram[:].rearrange("(a k) -> a k", a=1)
            )
            nc.gpsimd.dma_start(out=wob8, in_=wob8_dram[:])
            for h in range(8):
                nc.gpsimd.dma_start(
                    out=xb_sb[:, 2 * h : 2 * h + 2, :],
                    in_=xb_view[:, 2 * h : 2 * h + 2, :],
                )

            qk2_sb = proj.tile([K, 2, S], bf16)  # [:,0,:] q, [:,1,:] k
            v_sb = proj.tile([128, NT, 80], f8)  # cols 0:64 v, col 64 = 1/64
            nc.gpsimd.memset(v_sb[:, :, K : K + 1], 1.0 / AVS)
            avT8 = proj.tile([33, 2, S], f8)
            nc.gpsimd.memset(avT8[32:33, 0, :], AVS)
            nc.gpsimd.memset(avT8[32:33, 1, :], 0.0)
            util = proj.tile([128, 512], f32r)  # row 64: recip1 scratch

            exp_tiles = {}

            def emit_scores(tgt, pair_list):
                if tgt not in exp_tiles:
                    et = expp.tile([128, NT, 512], f8, tag="expT", name=f"expT{tgt}")
                    exp_tiles[tgt] = et
                et = exp_tiles[tgt]
                sqt = slice(tgt * 512, (tgt + 1) * 512)
                for p in pair_list:
                    pss = psS.tile([128, 2, 512], f32, tag="pss", name=f"pss{tgt}_{p}")
                    for j in range(2):
                        sk = 2 * p + j
                        nc.tensor.matmul(
                            pss[:, j, :],
                            qk2_sb[:, 1, sk * 128 : (sk + 1) * 128],
                            qk2_sb[:, 0, sqt],
                            start=True,
                            stop=True,
                        )
                    nc.scalar.activation(
                        out=et[:, 2 * p : 2 * p + 2, :], in_=pss[:],
                        func=AF.Exp, scale=0.125,
                    )

            # ---- phase 1: projections; block-0 scores piped in, block-1
            # scores emitted after (so expT(0) completes as early as possible
            # and phase 2 can start under the block-1 exp stream) ----
            with tc.tile_pool(name="psP", bufs=3, space="PSUM") as psP:
                for b in range(NB):
                    sq = slice(b * 512, (b + 1) * 512)
                    # q and k as separate 64-col chains, single-bank tiles
                    # (both land on partitions 0:64; DVE copies cannot cross
                    # partitions); biases folded in as ones-row matmuls.
                    # Block 0 interleaves the q/k chains across the two xt8
                    # DMA halves so PE starts as soon as half 1 lands.
                    psqk = [
                        psP.tile([K, 512], f32, tag=tg, bufs=1, name=f"ps{tg}")
                        for tg in ("psq", "psk")
                    ]
                    order = (
                        [(0, 0), (1, 0), (0, 1), (1, 1), (0, 2), (1, 2),
                         (0, 3), (1, 3)]
                        if b == 0
                        else [(j, cp) for j in range(2) for cp in range(4)]
                    )
                    for j, cp in order:
                        nc.tensor.matmul(
                            psqk[j],
                            wqkv8[:, 2 * cp : 2 * cp + 2, j * K : (j + 1) * K],
                            xt8_sb[:, 2 * cp : 2 * cp + 2, sq],
                            start=(cp == 0),
                            stop=False,
                            perf_mode=DR,
                        )
                    for j in range(2):
                        nc.tensor.matmul(
                            psqk[j],
                            bqk_row[:, j, :],
                            ones512r,
                            start=False,
                            stop=True,
                        )
                        nc.vector.tensor_copy(out=qk2_sb[:, j, sq], in_=psqk[j])
                    # scores for block 0 as its k-tiles become available;
                    # emitted before the v matmuls so the exp stream starts
                    # as early as possible
                    emit_scores(0, range(b * 2, b * 2 + 2))
                    # v in natural [s, j] layout: xt8 chunks stationary,
                    # wv8 chunk moving; bias via a ones-row matmul
                    psv = psP.tile([128, 4, K], f32, tag="psv", bufs=1)
                    for ti in range(4):
                        t = b * 4 + ti
                        for c in range(NC_):
                            nc.tensor.matmul(
                                psv[:, ti, :],
                                xt8_sb[:, c, t * 128 : (t + 1) * 128],
                                wqkv8[:, c, 128:192],
                                start=(c == 0),
                                stop=False,
                            )
                        nc.tensor.matmul(
                            psv[:, ti, :],
                            ones_row128,
                            bv_row8,
                            start=False,
                            stop=True,
                        )
                    nc.vector.tensor_copy(
                        out=v_sb[:, b * 4 : (b + 1) * 4, 0:K], in_=psv
                    )

            # ---- phase 2 ----
            out_view = out_dram[:].rearrange("(t p) d -> p t d", p=128)

            def emit_av(b, splits=1):
                """uav -> recip -> bcast -> av8 -> avT8 DMAs for block b.
                splits>1 pipelines the post-uav chain in sq-subslices so the
                first y-tiles of the block unblock after 1/splits of the
                chain (used for the cold block-0 seam)."""
                expT = exp_tiles.pop(b)
                psu = psU.tile([K + 1, 512], f32, tag="psu", name=f"psu{b}")
                for tp in range(NT // 2):
                    nc.tensor.matmul(
                        psu,
                        v_sb[:, 2 * tp : 2 * tp + 2, 0 : K + 1],
                        expT[:, 2 * tp : 2 * tp + 2, :],
                        start=(tp == 0),
                        stop=(tp == NT // 2 - 1),
                        perf_mode=DR,
                    )
                psbct = psS.tile([128, 2, 512], f32, tag="pss", name=f"psbc{b}")
                recipb = avn.tile([K, 512], f32, tag="recipb")
                av8 = avn.tile([K, 512], f8, tag="av8")
                w = 512 // splits
                for s in range(splits):
                    ss = slice(s * w, (s + 1) * w)
                    sq = slice(b * 512 + s * w, b * 512 + (s + 1) * w)
                    # recip = 64/sums at util row 64; broadcast to 64
                    # partitions via a f32r ones-matmul into a transient psS
                    # slot, then copy to SBUF so the av multiply reads only
                    # one PSUM operand
                    with nc.allow_low_precision(reason="recip feeds f32r bcast"):
                        nc.vector.reciprocal(
                            out=util[K : K + 1, ss], in_=psu[K : K + 1, ss]
                        )
                    psbc = psbct[0:K, 0, ss]
                    nc.tensor.matmul(
                        psbc,
                        ones_bc[K : K + 1, :],
                        util[K : K + 1, ss],
                        start=True,
                        stop=True,
                    )
                    nc.vector.tensor_copy(out=recipb[:, ss], in_=psbc)
                    nc.vector.tensor_tensor(
                        out=av8[:, ss], in0=psu[0:K, ss], in1=recipb[:, ss],
                        op=mybir.AluOpType.mult,
                    )
                    nc.sync.dma_start(out=avT8[0:32, 0, sq], in_=av8[0:32, ss])
                    nc.sync.dma_start(out=avT8[0:32, 1, sq], in_=av8[32:K, ss])
                if taps and b == 0:
                    nc.gpsimd.dma_start(out=tap_handles["T_RECB"][:], in_=recipb)

            if taps:
                nc.gpsimd.dma_start(out=tap_handles["T_QK"][:], in_=qk2_sb[:])
                nc.gpsimd.dma_start(out=tap_handles["T_V"][:], in_=v_sb[:, :, 0 : K + 1])
            # Software-pipelined LN: at iteration t, the j=1 half is normalized
            # on ACT (same engine as rstd, no cross-engine wait); the j=0 half
            # of iteration t-1 is normalized on DVE using the then-ready rstd,
            # so the in-order DVE queue never waits on ACT.
            #
            # Remaining score-pairs (blocks 1-3) are fed from a global queue,
            # 2 per tile iteration AFTER that tile's y-work, so the in-order
            # PE queue never parks y matmuls behind exp-paced score matmuls.
            pair_queue = [(tgt, p) for tgt in range(1, NB) for p in range(NT // 2)]

            def emit_next_pairs(n):
                for _ in range(n):
                    if pair_queue:
                        tgt, p = pair_queue.pop(0)
                        emit_scores(tgt, [p])

            with tc.tile_pool(name="psY", bufs=3, space="PSUM") as psY:
                if taps:
                    nc.gpsimd.dma_start(
                        out=tap_handles["T_EXP0"][:], in_=exp_tiles[0][:]
                    )
                emit_av(0, splits=2)
                emit_next_pairs(4)  # seam pre-fill while av(0) round-trips
                prev = None  # (psy0, out_sb, mv, rstd, t)
                for b in range(NB):
                    if taps and b == NB - 1:
                        nc.gpsimd.dma_start(out=tap_handles["T_AVT"][:], in_=avT8[:])
                    for ti in range(4):
                        t = b * 4 + ti
                        out_sb = outp.tile([128, D], bf16, tag="o")
                        psy = [None, None]
                        stats = work.tile([128, 2, 6], f32, tag="stats")
                        # block 3: the score-psum pool is free; use its 2-bank
                        # tiles for y so stats/norm run 1024-wide and the LN
                        # pipeline gets extra depth
                        psyt = None
                        if (b == NB - 1 and ti % 2 == 0) or t == 11:
                            psyt = psS.tile(
                                [128, 2, 512], f32, tag="pss", name=f"psy2_{t}"
                            )
                        for j in range(2):
                            if psyt is not None:
                                psy_j = psyt[:, j, :]
                            else:
                                psy_j = psY.tile([128, 512], f32, tag="ps")
                            psy[j] = psy_j
                            nc.tensor.matmul(
                                psy_j,
                                avT8[:, :, t * 128 : (t + 1) * 128],
                                wob8[:, :, j * 512 : (j + 1) * 512],
                                start=True,
                                stop=False,
                                perf_mode=DR,
                            )
                            nc.tensor.matmul(
                                psy_j,
                                ident512,
                                xb_sb[:, t, j * 512 : (j + 1) * 512],
                                start=False,
                                stop=True,
                            )
                            nc.vector.bn_stats(out=stats[:, j, :], in_=psy_j)
                        mv = work.tile([128, 2], f32, tag="mv")
                        nc.vector.bn_aggr(out=mv, in_=stats)
                        mneg = work.tile([128, 1], f32, tag="mneg")
                        nc.vector.tensor_scalar(
                            out=mneg, in0=mv[:, 0:1], scalar1=-1.0, scalar2=None,
                            op0=mybir.AluOpType.mult,
                        )
                        # rstd = (var+eps)^-0.5 = exp(-0.5*ln(var+eps)); Ln and
                        # Exp share act table 6, so no reloads vs softmax exp.
                        lnv = work.tile([128, 1], f32, tag="lnv")
                        nc.scalar.activation(
                            out=lnv, in_=mv[:, 1:2], func=AF.Ln,
                            bias=epsS_t, scale=1.0 / (YS * YS),
                        )
                        rstd = work.tile([128, 1], f32, tag="rstd")
                        nc.scalar.activation(
                            out=rstd, in_=lnv, func=AF.Exp, scale=-0.5,
                            bias=nln2_t,
                        )
                        # nm = -mu*rstd, on ACT so the chain stays ACT-local
                        nm = work.tile([128, 1], f32, tag="nm")
                        nc.scalar.activation(
                            out=nm, in_=mneg, func=AF.Copy, scale=rstd,
                        )
                        if psyt is None:
                            nc.scalar.activation(
                                out=out_sb[:, 512:1024], in_=psy[1],
                                func=AF.Identity, bias=nm, scale=rstd,
                            )
                            if t == NT - 1:
                                # last tile: store the j1 half as soon as it
                                # is normalized so its DMA pipeline overlaps
                                # the parallel j0 norm
                                nc.sync.dma_start(
                                    out=out_view[:, t, 512:1024],
                                    in_=out_sb[:, 512:1024],
                                )
                        if prev is not None:
                            p_psy0, p_out, p_mv, p_rstd, p_t = prev
                            nc.vector.tensor_scalar(
                                out=p_out[:, 0:512], in0=p_psy0,
                                scalar1=p_mv[:, 0:1], scalar2=p_rstd,
                                op0=mybir.AluOpType.subtract,
                                op1=mybir.AluOpType.mult,
                            )
                            nc.sync.dma_start(
                                out=out_view[:, p_t, :], in_=p_out
                            )
                            prev = None
                        if b == NB - 1 or psyt is not None:
                            # last block: exps are done, ACT has slack and DVE
                            # is the bottleneck -> norm j0 on ACT in-iteration
                            # (psS-based tiles normalize 1024-wide in one op,
                            # so skip the separate j1 norm below for them)
                            if psyt is not None:
                                nc.scalar.activation(
                                    out=out_sb[:].rearrange(
                                        "p (j d) -> p j d", j=2
                                    ),
                                    in_=psyt,
                                    func=AF.Identity, bias=nm, scale=rstd,
                                )
                            elif t >= NT - 3 and psyt is None:
                                # very last tile: nothing queued behind it on
                                # DVE, so run j0 there in parallel with ACT's
                                # j1, and store each half as soon as it is
                                # normalized (separate queues)
                                nc.vector.tensor_scalar(
                                    out=out_sb[:, 0:512], in0=psy[0],
                                    scalar1=mv[:, 0:1], scalar2=rstd,
                                    op0=mybir.AluOpType.subtract,
                                    op1=mybir.AluOpType.mult,
                                )
                            else:
                                nc.scalar.activation(
                                    out=out_sb[:, 0:512], in_=psy[0],
                                    func=AF.Identity, bias=nm, scale=rstd,
                                )
                            if t == NT - 1:
                                nc.sync.dma_start(
                                    out=out_view[:, t, 0:512],
                                    in_=out_sb[:, 0:512],
                                )
                            else:
                                nc.sync.dma_start(
                                    out=out_view[:, t, :], in_=out_sb
                                )
                        else:
                            prev = (psy[0], out_sb, mv, rstd, t)
                        if ti == 2 and b + 1 < NB:
                            emit_av(b + 1)
                        emit_next_pairs(2)
                # tail: finish any pending deferred tile
                if prev is not None:
                    p_psy0, p_out, p_mv, p_rstd, p_t = prev
                    nc.vector.tensor_scalar(
                        out=p_out[:, 0:512], in0=p_psy0,
                        scalar1=p_mv[:, 0:1], scalar2=p_rstd,
                        op0=mybir.AluOpType.subtract,
                        op1=mybir.AluOpType.mult,
                    )
                    nc.sync.dma_start(out=out_view[:, p_t, :], in_=p_out)

    nc.compile()
    return nc


def _get_compiled():
    if "nc" not in _COMPILED:
        _COMPILED["nc"] = _build_bass()
    return _COMPILED["nc"]


def _host_inputs(X, Wq, bq, Wk, bk, Wv, bv, Wo, bo):
    import ml_dtypes

    f8 = ml_dtypes.float8_e4m3
    bf = ml_dtypes.bfloat16
    f32 = np.float32

    # [D, 192] = [q|k|v] weights -> [128, 8, 192] with d = c*128 + p
    wqkv = np.concatenate([Wq, Wk, Wv], axis=1).astype(f32)
    wqkv8 = np.ascontiguousarray(
        wqkv.reshape(NC_, 128, 192).transpose(1, 0, 2)
    ).astype(f8)
    bqk = np.concatenate([bq, bk]).astype(f32)
    wob8 = np.zeros((33, 2, D), dtype=f8)
    wo8 = (Wo.astype(f32) * WOS).astype(f8)
    wob8[:32, 0, :] = wo8[0:32]
    wob8[:32, 1, :] = wo8[32:64]
    wob8[32, 0, :] = (bo.astype(f32) * WOS).astype(f8)

    common = {
        "WQKV8": wqkv8,
        "BQK": bqk,
        "BV": bv.astype(f32).astype(bf),
        "WOB8": wob8,
    }
    per_core = []
    for i in range(X.shape[0]):
        Xi = np.ascontiguousarray(X[i], dtype=f32)
        per_core.append(
            {
                "XB": Xi.astype(bf),
                "XT8": np.ascontiguousarray(Xi.T).astype(f8),
                **common,
            }
        )
    return per_core


def kernel(X, Wq, bq, Wk, bk, Wv, bv, Wo, bo, gamma, beta):
    from concourse.bass_utils import run_bass_kernel_spmd

    X = np.asarray(X, dtype=np.float32)
    gamma_np = np.asarray(gamma, dtype=np.float32)
    beta_np = np.asarray(beta, dtype=np.float32)

    nc = _get_compiled()
    in_maps = _host_inputs(
        X,
        np.asarray(Wq), np.asarray(bq), np.asarray(Wk), np.asarray(bk),
        np.asarray(Wv), np.asarray(bv), np.asarray(Wo), np.asarray(bo),
    )
    res = run_bass_kernel_spmd(nc, in_maps, core_ids=list(range(B)))
    out = np.stack(
        [np.asarray(res.results[i]["OUT"]).astype(np.float32) for i in range(B)],
        axis=0,
    )
    if not (np.all(gamma_np == 1.0) and np.all(beta_np == 0.0)):
        out = out * gamma_np + beta_np
    return out.astype(np.float32)

